# revision 13
# baseline (speedup 1.0000x reference)
"""Trainium2 Bass kernel for nn_BEMBFlex (within-category log-softmax utility model).

Strategy: shard ITEMS BY CATEGORY across the 8 cores. Categories are
rank-sorted by size and dealt round-robin (rank % 8 -> shard), so every
shard receives the same padded column layout and one SPMD program serves
all cores. Each core computes util = [th|ze] @ [alphaT; item_obsT] (+ lambda
via a K=1 accumulate-matmul) for all 1024 sessions over its ~1/8 of the
items, then does the within-category log-softmax locally (categories never
span shards). The host de-permutes the padded columns at the end.
"""

import sys

for _p in ("/opt/trn_rl_repo",):
    if _p not in sys.path:
        sys.path.insert(0, _p)

import ml_dtypes
import numpy as np

import concourse.bass as bass
import concourse.tile as tile
from concourse import bacc, bass_utils, mybir
from concourse.masks import make_identity

NUM_USERS = 100000
NUM_ITEMS = 25000
NUM_CATS = 500
LATENT = 64
BATCH = 1024
NCORES = 8
P = 128                    # partitions / sessions per matmul chunk
NCHUNKS = BATCH // P       # session chunks per core
BLOCK_COLS = 1536          # max padded cols per processing block (3 PSUM banks)
PAD_NEG = -1.0e30
SHIFT = 60.0               # constant exp shift; |util| < ~55 w.p. 1-1e-8, so
                           # exp(u-60) in [e^-140, e^-5]: no overflow, and every
                           # category sum stays >= e^-115 (no log(0)). The shift
                           # cancels exactly in u1 - ln(sum).

F32 = mybir.dt.float32
BF16 = mybir.dt.bfloat16
I32 = mybir.dt.int32

_nc_cache = {}


# ----------------------------------------------------------------------------
# Host-side layout
# ----------------------------------------------------------------------------

def _layout(cat_sizes):
    """Slot/block layout shared by all 8 shards.

    Categories sorted by size desc; slot i holds category ranks
    [8i, 8i+8) (one per shard). Slot width L_i = first (largest) size in
    the group rounded up to a multiple of 8. Blocks greedily group
    consecutive slots under a uniform L (the first slot's L) with
    g*L <= BLOCK_COLS.
    """
    order = np.argsort(-cat_sizes, kind="stable")
    order = order[cat_sizes[order] > 0]
    ncats = len(order)
    nslots = -(-ncats // NCORES)
    slot_L = np.empty(nslots, np.int64)
    for i in range(nslots):
        mx = int(cat_sizes[order[i * NCORES]])
        slot_L[i] = max(4, ((mx + 3) // 4) * 4)
    blocks = []  # (col0, g, L, slot0)
    col = 0
    i = 0
    while i < nslots:
        Lb = int(slot_L[i])
        g = 1
        while i + g < nslots and (g + 1) * Lb <= BLOCK_COLS:
            g += 1
        blocks.append((col, g, Lb, i))
        col += g * Lb
        i += g
    ipad = col
    slot_col = np.empty(nslots, np.int64)
    for (c0, g, Lb, s0) in blocks:
        for q in range(g):
            slot_col[s0 + q] = c0 + q * Lb
    return order, blocks, ipad, slot_col


def _prep(inputs):
    cat = np.asarray(inputs["category_idx"]).astype(np.int64).ravel()
    cat_sizes = np.bincount(cat, minlength=NUM_CATS)
    order, blocks, ipad, slot_col = _layout(cat_sizes)

    rank = np.full(NUM_CATS, -1, np.int64)
    rank[order] = np.arange(len(order))

    # position of each item within its category (stable order)
    perm = np.argsort(cat, kind="stable")
    starts = np.searchsorted(cat[perm], np.arange(NUM_CATS))
    within_sorted = np.arange(NUM_ITEMS) - starts[cat[perm]]
    item_within = np.empty(NUM_ITEMS, np.int64)
    item_within[perm] = within_sorted

    r = rank[cat]
    item_shard = r % NCORES
    item_col = slot_col[r // NCORES] + item_within

    alpha = np.ascontiguousarray(np.asarray(inputs["alpha_item"], np.float32))
    obs = np.ascontiguousarray(np.asarray(inputs["item_obs"], np.float32))
    lam = np.asarray(inputs["lambda_item"], np.float32).ravel()

    W = np.zeros((NCORES, 2 * LATENT, ipad), np.float32)
    LAM = np.full((NCORES, 1, ipad), PAD_NEG, np.float32)
    for s in range(NCORES):
        m = item_shard == s
        cols = item_col[m]
        W[s, 0:LATENT, cols] = alpha[m]
        W[s, LATENT:, cols] = obs[m]
        LAM[s, 0, cols] = lam[m]
    W = W.astype(ml_dtypes.bfloat16)
    LAM = LAM.astype(ml_dtypes.bfloat16)

    uidx = np.asarray(inputs["user_index"]).astype(np.int64).ravel()
    theta = np.asarray(inputs["theta_user"], np.float32)
    zeta = np.asarray(inputs["zeta_user"], np.float32)
    # [2K, B] pre-gathered, pre-transposed session features (host-side shard prep)
    thzet = np.ascontiguousarray(
        np.concatenate([theta[uidx], zeta[uidx]], axis=1).T
    ).astype(ml_dtypes.bfloat16)
    return {
        "blocks": blocks,
        "ipad": ipad,
        "item_shard": item_shard,
        "item_col": item_col,
        "W": W,
        "LAM": LAM,
        "thzet": thzet,
    }


# ----------------------------------------------------------------------------
# Device program
# ----------------------------------------------------------------------------

def _bcast3(t2d, L):
    """[P, g] tile -> [P, g, L] read-AP with step-0 innermost broadcast."""
    ap = t2d[:, :]
    return bass.AP(tensor=ap.tensor, offset=ap.offset, ap=[*ap.ap, [0, L]])


def _build_nc(blocks, ipad):
    nc = bacc.Bacc(
        "TRN2",
        debug=False,
        enable_asserts=False,
        target_bir_lowering=False,
        num_devices=NCORES,
    )
    w_d = nc.dram_tensor("W", [2 * LATENT, ipad], BF16, kind="ExternalInput").ap()
    lam_d = nc.dram_tensor("LAM", [1, ipad], BF16, kind="ExternalInput").ap()
    thzet_d = nc.dram_tensor("THZET", [2 * LATENT, BATCH], BF16, kind="ExternalInput").ap()
    out_d = nc.dram_tensor("O", [BATCH, ipad], F32, kind="ExternalOutput").ap()

    gtot = sum(g for (_c, g, _l, _s) in blocks)
    with tile.TileContext(nc) as tc:
        with (
            tc.tile_pool(name="singles", bufs=1) as singles,
            tc.tile_pool(name="psum_u", bufs=2, space="PSUM") as psum_u,
            tc.tile_pool(name="ubuf", bufs=2 * len(blocks)) as ubuf,
            tc.tile_pool(name="exbuf", bufs=3) as exbuf,
            tc.tile_pool(name="stats", bufs=3) as stats,
        ):
            w_sb = singles.tile([2 * LATENT, ipad], BF16, name="w_sb")
            nc.sync.dma_start(out=w_sb[:, :], in_=w_d[:, :])
            lam_sb = singles.tile([1, ipad], BF16, name="lam_sb")
            nc.sync.dma_start(out=lam_sb[:, :], in_=lam_d[:, :])
            ones_sb = singles.tile([1, P], BF16, name="ones_sb")
            nc.vector.memset(ones_sb[:, :], 1.0)
            thzet_sb = singles.tile([2 * LATENT, BATCH], BF16, name="thzet_sb")
            nc.sync.dma_start(out=thzet_sb[:, :], in_=thzet_d[:, :])
            thze_t = [thzet_sb[:, j * P:(j + 1) * P] for j in range(NCHUNKS)]

            for j in range(NCHUNKS):
                u1s = []
                s_j = stats.tile([P, gtot], F32, name="s_j", tag="s_j")
                goff = 0
                for (col0, g, L, _s0) in blocks:
                    cols = g * L
                    up = psum_u.tile([P, cols], F32, name="up", tag="up")
                    for c0 in range(0, cols, 512):
                        cn = min(512, cols - c0)
                        nc.tensor.matmul(
                            up[:, c0:c0 + cn],
                            lhsT=thze_t[j],
                            rhs=w_sb[:, col0 + c0:col0 + c0 + cn],
                            start=True,
                            stop=False,
                        )
                        nc.tensor.matmul(
                            up[:, c0:c0 + cn],
                            lhsT=ones_sb[:, :],
                            rhs=lam_sb[:, col0 + c0:col0 + c0 + cn],
                            start=False,
                            stop=True,
                        )
                    # evict PSUM with the constant shift: u1 = u - SHIFT
                    u1 = ubuf.tile([P, cols], BF16 if False else F32, name="u1", tag="u1")
                    nc.vector.tensor_scalar(
                        out=u1[:, :],
                        in0=up[:, :],
                        scalar1=SHIFT,
                        scalar2=None,
                        op0=mybir.AluOpType.subtract,
                    )
                    u1s.append(u1)
                    ex = exbuf.tile([P, cols], BF16, name="ex", tag="ex")
                    nc.scalar.activation(
                        out=ex[:, :], in_=u1[:, :], func=mybir.ActivationFunctionType.Exp
                    )
                    nc.vector.reduce_sum(
                        out=s_j[:, goff:goff + g],
                        in_=ex[:, :].rearrange("p (g l) -> p g l", l=L),
                        axis=mybir.AxisListType.X,
                    )
                    goff += g
                ls_j = stats.tile([P, gtot], F32, name="ls_j", tag="ls_j")
                nc.scalar.activation(
                    out=ls_j[:, :], in_=s_j[:, :], func=mybir.ActivationFunctionType.Ln
                )
                goff = 0
                for bi, (col0, g, L, _s0) in enumerate(blocks):
                    cols = g * L
                    u1 = u1s[bi]
                    ls_b = _bcast3(ls_j[:, goff:goff + g], L)
                    nc.gpsimd.tensor_tensor(
                        out=u1[:, :].rearrange("p (g l) -> p g l", l=L),
                        in0=u1[:, :].rearrange("p (g l) -> p g l", l=L),
                        in1=ls_b,
                        op=mybir.AluOpType.subtract,
                    )
                    nc.sync.dma_start(
                        out=out_d[j * P:(j + 1) * P, col0:col0 + cols], in_=u1[:, :]
                    )
                    goff += g
    nc.compile()
    return nc


# ----------------------------------------------------------------------------
# Entry points
# ----------------------------------------------------------------------------

def run(inputs, trace=False):
    prep = _prep(inputs)
    key = (prep["ipad"], tuple(prep["blocks"]))
    nc = _nc_cache.get(key)
    if nc is None:
        nc = _build_nc(prep["blocks"], prep["ipad"])
        _nc_cache[key] = nc
    in_maps = [
        {
            "W": prep["W"][c],
            "LAM": prep["LAM"][c],
            "THZET": prep["thzet"],
        }
        for c in range(NCORES)
    ]
    res = bass_utils.run_bass_kernel_spmd(
        nc, in_maps, core_ids=list(range(NCORES)), trace=trace
    )
    big = np.stack([res.results[c]["O"] for c in range(NCORES)])  # [8, B, ipad]
    out = np.ascontiguousarray(
        big[prep["item_shard"], :, prep["item_col"]].T
    ).astype(np.float32)
    return out, res


def kernel(**inputs) -> np.ndarray:
    out, _ = run(inputs, trace=False)
    return out
